# revision 1
# baseline (speedup 1.0000x reference)
"""Sparse (sliding-window) attention head on 8 TRN2 NeuronCores.

Reference computation (B=2, S=4096, D=512, HD=64, SCALE=128):
    q = x @ wq ; k = x @ wk ; v = x @ wv          [B,S,64]
    scores[b,s,w] = q[b,s] . k[b,s-128+w] / 8     w in [0,256), zero-padded OOB
    out = softmax_w(scores) @ v_window            [B,S,64]

Sharding: 8 shards = (batch b, 1024-seq chunk c). Each shard gets a
zero-padded 128-halo of x on both sides, which reproduces the reference's
zero-padded (not masked) window semantics exactly. All compute is local,
no collectives.

Device layout (per core):
    xT   [512,1280] bf16  host-pre-transposed padded input shard
    w3   [128,3,4,64] bf16  packed wq|wk|wv (d-chunk on partitions)
    mask [128,384] bf16  band-validity mask for one 128-query block
    out  [1024,64] f32

    qT,kT = w.T @ xT   (head dim on partitions)
    v     = xT.T @ wv  (natural layout, keys on partitions) + ones column
    per 128-query block qb:
        scT[key,que] = kT_chunk.T @ qT_block      3 chunks of [128,128]
        e = exp(scT/8) * mask                     bf16
        av[que,0:65] = sum_c e_c.T @ vaug_c       ones col -> softmax denom
        out_block = av[:, :64] * (1/av[:, 64])
"""

import sys
import types

import numpy as np
import ml_dtypes

B, S, D = 2, 4096, 512
HD = 64
SCALE = 128
SS = S // 4          # 1024 positions per shard
HP = SCALE           # halo padding each side
NP = SS + 2 * HP     # 1280 padded positions
NKC = NP // 128      # 10 key chunks
NQB = SS // 128      # 8 query blocks
NDC = D // 128       # 4 d-chunks

_CACHE = {}


def _ensure_hooks():
    """Register the axon NTFF profile hook; keep artifacts local."""
    if "antenv.axon_hooks" not in sys.modules:
        try:
            from trn_agent_boot.trn_boot import _ntff_profile_via_ctypes

            m = types.ModuleType("antenv.axon_hooks")
            m.get_axon_ntff_profile_hook = lambda: _ntff_profile_via_ctypes(
                "/opt/axon/libaxon_pjrt.so"
            )
            sys.modules["antenv.axon_hooks"] = m
        except Exception:
            pass
    import concourse.bass_utils as bass_utils

    bass_utils.upload_artifacts = lambda tmpdir: tmpdir


def _build_nc():
    import concourse.mybir as mybir
    import concourse.tile as tile
    from concourse import bacc

    bf = mybir.dt.bfloat16
    f32 = mybir.dt.float32
    AF = mybir.ActivationFunctionType

    nc = bacc.Bacc("TRN2", target_bir_lowering=False, debug=False, num_devices=8)

    xT_d = nc.dram_tensor("xT", [D, NP], bf, kind="ExternalInput")
    w3_d = nc.dram_tensor("w3", [128, 3, NDC, HD], bf, kind="ExternalInput")
    mask_d = nc.dram_tensor("mask", [128, 384], bf, kind="ExternalInput")
    out_d = nc.dram_tensor("out", [SS, HD], f32, kind="ExternalOutput")

    with tile.TileContext(nc) as tc:
        with (
            tc.tile_pool(name="consts", bufs=1) as consts,
            tc.tile_pool(name="xtp", bufs=1) as xtp,
            tc.tile_pool(name="qkp", bufs=1) as qkp,
            tc.tile_pool(name="vgp", bufs=1) as vgp,
            tc.tile_pool(name="work", bufs=3) as work,
            tc.tile_pool(name="fin", bufs=3) as fin,
        ):
            w_s = consts.tile([128, 3, NDC, HD], bf)
            nc.sync.dma_start(out=w_s, in_=w3_d[:, :, :, :])
            mask_s = consts.tile([128, 384], bf)
            nc.sync.dma_start(out=mask_s, in_=mask_d[:, :])

            # Trigger the ACT exp table load early so it hides under DMA/proj.
            zz = consts.tile([128, 1], f32)
            nc.vector.memset(zz, 0.0)
            ez = consts.tile([128, 1], f32)
            nc.scalar.activation(ez, zz, AF.Exp)

            # Few big DMAs: each HWDGE dma_start costs ~600ns of serial
            # issue time on the Sync sequencer.
            xt = []
            for dc in range(NDC):
                t = xtp.tile([128, NP], bf, tag=f"xt{dc}")
                nc.sync.dma_start(out=t, in_=xT_d[dc * 128 : (dc + 1) * 128, :])
                xt.append(t)

            # DMA-free garbage tile for PE warmup.
            garb = consts.tile([128, 512], bf)
            nc.vector.memset(garb, 0.5)

            qT_s = qkp.tile([64, SS], bf, tag="qT")
            kT_s = qkp.tile([64, NP], bf, tag="kT")
            vaug = vgp.tile([128, NKC, 66], bf)
            nc.vector.memset(vaug[:, :, 64:66], 1.0)

            with (
                tc.tile_pool(name="wrm", bufs=1, space="PSUM") as wrm,
                tc.tile_pool(name="pps", bufs=3, space="PSUM") as pps,
                tc.tile_pool(name="vps", bufs=3, space="PSUM") as vps,
            ):
                # PE warmup: dummy matmuls on the memset tile while the xT
                # DMAs land, so HAM un-throttles (1.2 -> 2.4 GHz) before the
                # real work reaches the array.
                wps = wrm.tile([64, 512], f32, tag="warm")
                for _ in range(7):
                    nc.tensor.matmul(
                        wps,
                        lhsT=garb[:, 0:64],
                        rhs=garb[:, :],
                        start=True,
                        stop=True,
                    )

                # qT / kT projections: head dim on partitions. kT evacs on
                # ACT, qT on DVE to split the PSUM-drain load.
                segs_q = [(0, 512), (512, 512)]
                segs_k = [(0, 512), (512, 512), (1024, 256)]
                for j, dst, off, segs in ((1, kT_s, 0, segs_k), (0, qT_s, HP, segs_q)):
                    for s0, w in segs:
                        ps = pps.tile([64, 512], f32, tag="pp")
                        for dc in range(NDC):
                            nc.tensor.matmul(
                                ps[:, :w],
                                lhsT=w_s[:, j, dc, :],
                                rhs=xt[dc][:, off + s0 : off + s0 + w],
                                start=(dc == 0),
                                stop=(dc == NDC - 1),
                            )
                        if j == 1:
                            nc.scalar.copy(dst[:, s0 : s0 + w], ps[:, :w])
                        else:
                            nc.vector.tensor_copy(dst[:, s0 : s0 + w], ps[:, :w])

                # v in natural layout (keys on partitions) + ones column.
                for kc in range(NKC):
                    vp = vps.tile([128, HD], f32, tag="vp")
                    for dc in range(NDC):
                        nc.tensor.matmul(
                            vp,
                            lhsT=xt[dc][:, kc * 128 : (kc + 1) * 128],
                            rhs=w_s[:, 2, dc, :],
                            start=(dc == 0),
                            stop=(dc == NDC - 1),
                        )
                    nc.vector.tensor_copy(vaug[:, kc, 0:64], vp)

            with (
                tc.tile_pool(name="sps", bufs=3, space="PSUM") as sps,
                tc.tile_pool(name="aps", bufs=3, space="PSUM") as aps,
            ):
                for qb in range(NQB):
                    sc = sps.tile([128, 384], f32, tag="sc")
                    for c in range(3):
                        nc.tensor.matmul(
                            sc[:, c * 128 : (c + 1) * 128],
                            lhsT=kT_s[:, (qb + c) * 128 : (qb + c + 1) * 128],
                            rhs=qT_s[:, qb * 128 : (qb + 1) * 128],
                            start=True,
                            stop=True,
                        )
                    ex = work.tile([128, 384], bf, tag="ex")
                    nc.scalar.activation(ex, sc, AF.Exp, scale=0.125)
                    em = work.tile([128, 384], bf, tag="em")
                    nc.vector.tensor_mul(em, ex, mask_s)
                    av = aps.tile([128, 65], f32, tag="av")
                    for c in range(3):
                        nc.tensor.matmul(
                            av,
                            lhsT=em[:, c * 128 : (c + 1) * 128],
                            rhs=vaug[:, qb + c, 0:65],
                            start=(c == 0),
                            stop=(c == 2),
                        )
                    rc = fin.tile([128, 1], f32, tag="rc")
                    nc.vector.reciprocal(rc, av[:, 64:65])
                    ot = fin.tile([128, HD], f32, tag="ot")
                    nc.vector.tensor_scalar_mul(ot, av[:, 0:HD], rc)
                    nc.sync.dma_start(
                        out=out_d[qb * 128 : (qb + 1) * 128, :], in_=ot
                    )

    nc.compile()
    return nc


def _get_nc():
    if "nc" not in _CACHE:
        _ensure_hooks()
        _CACHE["nc"] = _build_nc()
    return _CACHE["nc"]


def _host_inputs(inputs, wq, wk, wv):
    bf16 = ml_dtypes.bfloat16
    x = np.asarray(inputs, dtype=np.float32)

    # w3[p, j, c, m] = w_j[c*128 + p, m]
    w3 = np.stack([wq, wk, wv]).astype(np.float32)          # [3, 512, 64]
    w3 = w3.reshape(3, NDC, 128, HD).transpose(2, 0, 1, 3)   # [128, 3, 4, 64]
    w3 = np.ascontiguousarray(w3).astype(bf16)

    p = np.arange(128)[:, None]
    q = np.arange(128)[None, :]
    mask = np.concatenate(
        [(p >= q), np.ones((128, 128), bool), (p < q)], axis=1
    ).astype(bf16)                                           # [128, 384]

    in_maps = []
    for i in range(8):
        b, c = divmod(i, 4)
        s0 = c * SS
        xp = np.zeros((NP, D), np.float32)
        lo = max(0, s0 - HP)
        hi = min(S, s0 + SS + HP)
        xp[lo - (s0 - HP) : hi - (s0 - HP)] = x[b, lo:hi]
        xT = np.ascontiguousarray(xp.T).astype(bf16)         # [512, 1280]
        in_maps.append({"xT": xT, "w3": w3, "mask": mask})
    return in_maps


def run_sharded(inputs, wq, wk, wv, trace=False, trace_cores=None):
    """Run the SPMD kernel; returns (out [B,S,HD] f32, BassKernelResults)."""
    _ensure_hooks()
    import concourse.bass_utils as bass_utils

    nc = _get_nc()
    in_maps = _host_inputs(inputs, wq, wk, wv)
    res = bass_utils.run_bass_kernel_spmd(
        nc,
        in_maps,
        core_ids=list(range(8)),
        trace=trace,
        trace_cores=trace_cores,
    )
    out = np.empty((B, S, HD), np.float32)
    for i in range(8):
        b, c = divmod(i, 4)
        out[b, c * SS : (c + 1) * SS] = res.results[i]["out"]
    return out, res


def kernel(inputs, wq, wk, wv):
    out, _ = run_sharded(inputs, wq, wk, wv, trace=False)
    return out



# revision 8
# speedup vs baseline: 1.1046x; 1.1046x over previous
"""Sparse (sliding-window) attention head on 8 TRN2 NeuronCores.

Reference computation (B=2, S=4096, D=512, HD=64, SCALE=128):
    q = x @ wq ; k = x @ wk ; v = x @ wv          [B,S,64]
    scores[b,s,w] = q[b,s] . k[b,s-128+w] / 8     w in [0,256), zero-padded OOB
    out = softmax_w(scores) @ v_window            [B,S,64]

Sharding: 8 shards = (batch b, 1024-seq chunk c). Each shard gets a
zero-padded 128-halo of x on both sides, which reproduces the reference's
zero-padded (not masked) window semantics exactly. All compute is local.

v2 layout (per core):
    x arrives as 4 column-block tiles [128, 4dc, w] (bf16, host-transposed),
    DMA'd in consumption order on both HWDGE engines (Sync + Scalar).
    wq|wk packed into one [128,128] lhsT per d-chunk -> one PE pass emits
    qT (rows 0:64) and kT (rows 64:128) together into qkT [128, 1280].
    v stays natural [key,64] via per-chunk matmuls, packed pairwise in PSUM.
    Attention per 128-query block qb (starts as soon as its 384-col window
    of kT/qT/v is evacuated):
        scT[key,que] = kT_chunk.T @ qT_block      3 chunks of [128,128]
        ex = exp(scT/8) (ACT), em = ex*mask (DVE/Pool alternating)
        av4[:, qb%4, 0:65] += em_c.T @ vaug_c     ones col -> softmax denom
        norm: recip batched per 4 blocks, out_block = av*(1/denom)
    Output [128, 8, 64] partition-major, 2 DMAs; host unshuffles.
"""

import sys
import types

import numpy as np
import ml_dtypes

B, S, D = 2, 4096, 512
HD = 64
SCALE = 128
SS = S // 4          # 1024 positions per shard
HP = SCALE           # halo padding each side
NP = SS + 2 * HP     # 1280 padded positions
NKC = NP // 128      # 10 key chunks
NQB = SS // 128      # 8 query blocks
NDC = D // 128       # 4 d-chunks

# x column blocks (multiples of 128; qk proj segments == blocks)
BLOCKS = [(0, 384), (384, 384), (768, 256), (1024, 256)]

_CACHE = {}


def _ensure_hooks():
    """Register the axon NTFF profile hook; keep artifacts local."""
    if "antenv.axon_hooks" not in sys.modules:
        try:
            from trn_agent_boot.trn_boot import _ntff_profile_via_ctypes

            m = types.ModuleType("antenv.axon_hooks")
            m.get_axon_ntff_profile_hook = lambda: _ntff_profile_via_ctypes(
                "/opt/axon/libaxon_pjrt.so"
            )
            sys.modules["antenv.axon_hooks"] = m
        except Exception:
            pass
    import concourse.bass_utils as bass_utils

    bass_utils.upload_artifacts = lambda tmpdir: tmpdir


def _build_nc():
    import concourse.mybir as mybir
    import concourse.tile as tile
    from concourse import bacc

    bf = mybir.dt.bfloat16
    f32 = mybir.dt.float32
    AF = mybir.ActivationFunctionType

    nc = bacc.Bacc("TRN2", target_bir_lowering=False, debug=False, num_devices=8)

    x_d = nc.dram_tensor("x4", [128, NDC, NP], bf, kind="ExternalInput")
    w_d = nc.dram_tensor("wqkv", [128, NDC, 192], bf, kind="ExternalInput")
    mask_d = nc.dram_tensor("mask", [128, 384], bf, kind="ExternalInput")
    out_d = nc.dram_tensor("outp", [128, NQB, HD], f32, kind="ExternalOutput")

    # chunk -> block containing its 128 cols
    def blk_of(col):
        for bi, (s0, w) in enumerate(BLOCKS):
            if s0 <= col < s0 + w:
                return bi, col - s0
        raise AssertionError(col)

    with tile.TileContext(nc) as tc:
        with (
            tc.tile_pool(name="consts", bufs=1) as consts,
            tc.tile_pool(name="xtp", bufs=1) as xtp,
            tc.tile_pool(name="qkp", bufs=1) as qkp,
            tc.tile_pool(name="vgp", bufs=1) as vgp,
            tc.tile_pool(name="exp_p", bufs=2) as exp_p,
            tc.tile_pool(name="emp", bufs=2) as emp,
            tc.tile_pool(name="fin", bufs=2) as fin,
            tc.tile_pool(name="qkps", bufs=2, space="PSUM") as qkps,
            tc.tile_pool(name="vps", bufs=2, space="PSUM") as vps,
            tc.tile_pool(name="scps", bufs=2, space="PSUM") as scps,
            tc.tile_pool(name="avps", bufs=2, space="PSUM") as avps,
        ):
            # ---- DMAs first, split across both HWDGE engines ----
            xt = []
            for bi, (s0, w) in enumerate(BLOCKS):
                t = xtp.tile([128, NDC, w], bf, tag=f"xt{bi}")
                xt.append(t)
            w_s = consts.tile([128, NDC, 192], bf, tag="w")
            mask_s = consts.tile([128, 384], bf, tag="mask")

            nc.sync.dma_start(out=xt[0], in_=x_d[:, :, 0:384])
            nc.scalar.dma_start(out=w_s, in_=w_d[:, :, :])
            nc.sync.dma_start(out=xt[1], in_=x_d[:, :, 384:768])
            nc.scalar.dma_start(out=mask_s, in_=mask_d[:, :])
            nc.sync.dma_start(out=xt[2], in_=x_d[:, :, 768:1024])
            nc.sync.dma_start(out=xt[3], in_=x_d[:, :, 1024:1280])

            # ---- memsets + ACT exp-table trigger ----
            zz = consts.tile([128, 1], f32, tag="zz")
            nc.gpsimd.memset(zz, 0.0)
            garb = consts.tile([128, 260], bf, tag="garb")
            nc.gpsimd.memset(garb, 0.5)
            vaug = vgp.tile([128, NKC, 66], bf, tag="vaug")
            nc.gpsimd.memset(vaug[:, :, 64:66], 1.0)
            ez = consts.tile([128, 1], f32, tag="ez")
            nc.scalar.activation(ez, zz, AF.Exp)

            qT_s = qkp.tile([64, SS], bf, tag="qT")
            kT_s = qkp.tile([64, NP], bf, tag="kT")
            ot = fin.tile([128, NQB, HD], f32, tag="ot")

            # ---- PE warmup: ramp to full clock while DMAs land ----
            for i in range(16):
                wp = avps.tile([128, 4, 65], f32, tag="av4")
                nc.tensor.matmul(
                    wp[:, :, :],
                    lhsT=garb[:, 0:128],
                    rhs=garb[:, :],
                    start=True,
                    stop=True,
                )

            # ---- helpers ----
            def qk_seg(bi):
                s0, w = BLOCKS[bi]
                ps = qkps.tile([128, 384], f32, tag="qkps")
                for dc in range(NDC):
                    nc.tensor.matmul(
                        ps[:, :w],
                        lhsT=w_s[:, dc, 0:128],
                        rhs=xt[bi][:, dc, :],
                        start=(dc == 0),
                        stop=(dc == NDC - 1),
                    )
                return ps, s0, w

            def v_chunk(kc, vp, j):
                bi, off = blk_of(kc * 128)
                for dc in range(NDC):
                    nc.tensor.matmul(
                        vp[:, j, :],
                        lhsT=xt[bi][:, dc, off : off + 128],
                        rhs=w_s[:, dc, 128:192],
                        start=(dc == 0),
                        stop=(dc == NDC - 1),
                    )

            def evac_seg(ps, s0, w, eng):
                # k rows (shifted down 64 partitions) + valid q cols
                cp = eng.copy if eng is nc.scalar else eng.tensor_copy
                cp(kT_s[:, s0 : s0 + w], ps[64:128, :w])
                qa, qb_ = max(s0, HP), min(s0 + w, HP + SS)
                if qa < qb_:
                    cp(qT_s[:, qa - HP : qb_ - HP], ps[0:64, qa - s0 : qb_ - s0])

            def sc_block(qb):
                sc = scps.tile([128, 384], f32, tag="sc")
                for c in range(3):
                    nc.tensor.matmul(
                        sc[:, c * 128 : (c + 1) * 128],
                        lhsT=kT_s[:, (qb + c) * 128 : (qb + c + 1) * 128],
                        rhs=qT_s[:, qb * 128 : (qb + 1) * 128],
                        start=True,
                        stop=True,
                    )
                return sc

            def exp_mask(qb, sc):
                ex = exp_p.tile([128, 384], bf, tag="ex")
                nc.scalar.activation(ex, sc, AF.Exp, scale=0.125)
                em = emp.tile([128, 384], bf, tag="em")
                nc.gpsimd.tensor_mul(em, ex, mask_s)
                return em

            def av_block(qb, em, av4, j):
                for c in range(3):
                    nc.tensor.matmul(
                        av4[:, j, :],
                        lhsT=em[:, c * 128 : (c + 1) * 128],
                        rhs=vaug[:, qb + c, 0:65],
                        start=(c == 0),
                        stop=(c == 2),
                    )

            # ---- pipeline ----
            # seg0 (cols 0:384) + v chunks 0,1,2
            ps0, s0, w0 = qk_seg(0)
            evac_seg(ps0, s0, w0, nc.scalar)
            vp01 = vps.tile([128, 2, HD], f32, tag="vp")
            v_chunk(0, vp01, 0)
            v_chunk(1, vp01, 1)
            nc.vector.tensor_copy(vaug[:, 0:2, 0:64], vp01)
            vp23 = vps.tile([128, 2, HD], f32, tag="vp")
            v_chunk(2, vp23, 0)

            # seg1 (cols 384:768) + v chunk 3
            ps1, s1, w1 = qk_seg(1)
            evac_seg(ps1, s1, w1, nc.vector)
            v_chunk(3, vp23, 1)
            nc.vector.tensor_copy(vaug[:, 2:4, 0:64], vp23)

            # qb0, qb1
            av4a = avps.tile([128, 4, 65], f32, tag="av4")
            sc0 = sc_block(0)
            em0 = exp_mask(0, sc0)
            av_block(0, em0, av4a, 0)
            sc1 = sc_block(1)
            em1 = exp_mask(1, sc1)
            av_block(1, em1, av4a, 1)

            # v chunks 4,5 (block B), seg2 (cols 768:1024)
            vp45 = vps.tile([128, 2, HD], f32, tag="vp")
            v_chunk(4, vp45, 0)
            v_chunk(5, vp45, 1)
            nc.vector.tensor_copy(vaug[:, 4:6, 0:64], vp45)
            ps2, s2, w2 = qk_seg(2)
            evac_seg(ps2, s2, w2, nc.vector)

            # qb2, qb3
            sc2 = sc_block(2)
            em2 = exp_mask(2, sc2)
            av_block(2, em2, av4a, 2)
            sc3 = sc_block(3)
            em3 = exp_mask(3, sc3)
            av_block(3, em3, av4a, 3)

            # v chunks 6,7 (block C1), seg3 (cols 1024:1280), v chunks 8,9
            vp67 = vps.tile([128, 2, HD], f32, tag="vp")
            v_chunk(6, vp67, 0)
            v_chunk(7, vp67, 1)
            nc.vector.tensor_copy(vaug[:, 6:8, 0:64], vp67)
            ps3, s3, w3 = qk_seg(3)
            evac_seg(ps3, s3, w3, nc.vector)
            vp89 = vps.tile([128, 2, HD], f32, tag="vp")
            v_chunk(8, vp89, 0)
            v_chunk(9, vp89, 1)
            nc.vector.tensor_copy(vaug[:, 8:10, 0:64], vp89)

            # group0 normalize + first output DMA
            rc0 = fin.tile([128, 4], f32, tag="rc")
            nc.vector.reciprocal(rc0, av4a[:, :, 64])
            for j in range(4):
                nc.vector.tensor_scalar_mul(
                    ot[:, j, :], av4a[:, j, 0:64], rc0[:, j : j + 1]
                )
            nc.sync.dma_start(out=out_d[:, 0:4, :], in_=ot[:, 0:4, :])

            # qb4..qb7
            av4b = avps.tile([128, 4, 65], f32, tag="av4")
            sc4 = sc_block(4)
            em4 = exp_mask(4, sc4)
            av_block(4, em4, av4b, 0)
            sc5 = sc_block(5)
            em5 = exp_mask(5, sc5)
            av_block(5, em5, av4b, 1)
            sc6 = sc_block(6)
            em6 = exp_mask(6, sc6)
            av_block(6, em6, av4b, 2)
            sc7 = sc_block(7)
            em7 = exp_mask(7, sc7)
            av_block(7, em7, av4b, 3)

            # group1 normalize + second output DMA
            rc1 = fin.tile([128, 4], f32, tag="rc")
            nc.vector.reciprocal(rc1, av4b[:, :, 64])
            for j in range(4):
                nc.scalar.activation(
                    ot[:, 4 + j, :],
                    av4b[:, j, 0:64],
                    AF.Copy,
                    scale=rc1[:, j : j + 1],
                )
            nc.sync.dma_start(out=out_d[:, 4:8, :], in_=ot[:, 4:8, :])

    nc.compile()
    return nc


def _get_nc():
    if "nc" not in _CACHE:
        _ensure_hooks()
        _CACHE["nc"] = _build_nc()
    return _CACHE["nc"]


def _host_inputs(inputs, wq, wk, wv):
    bf16 = ml_dtypes.bfloat16
    x = np.asarray(inputs, dtype=np.float32)

    # wqkv[p, dc, 0:64]=wq, [64:128]=wk, [128:192]=wv  (rows dc*128+p)
    wcat = np.concatenate(
        [np.asarray(wq), np.asarray(wk), np.asarray(wv)], axis=1
    ).astype(np.float32)                                     # [512, 192]
    wqkv = np.ascontiguousarray(
        wcat.reshape(NDC, 128, 192).transpose(1, 0, 2)
    ).astype(bf16)                                           # [128, 4, 192]

    p = np.arange(128)[:, None]
    q = np.arange(128)[None, :]
    mask = np.concatenate(
        [(p >= q), np.ones((128, 128), bool), (p < q)], axis=1
    ).astype(bf16)                                           # [128, 384]

    in_maps = []
    for i in range(8):
        b, c = divmod(i, 4)
        s0 = c * SS
        xp = np.zeros((NP, D), np.float32)
        lo = max(0, s0 - HP)
        hi = min(S, s0 + SS + HP)
        xp[lo - (s0 - HP) : hi - (s0 - HP)] = x[b, lo:hi]
        x4 = np.ascontiguousarray(
            xp.T.reshape(NDC, 128, NP).transpose(1, 0, 2)
        ).astype(bf16)                                       # [128, 4, 1280]
        in_maps.append({"x4": x4, "wqkv": wqkv, "mask": mask})
    return in_maps


def run_sharded(inputs, wq, wk, wv, trace=False, trace_cores=None):
    """Run the SPMD kernel; returns (out [B,S,HD] f32, BassKernelResults)."""
    _ensure_hooks()
    import concourse.bass_utils as bass_utils

    nc = _get_nc()
    in_maps = _host_inputs(inputs, wq, wk, wv)
    res = bass_utils.run_bass_kernel_spmd(
        nc,
        in_maps,
        core_ids=list(range(8)),
        trace=trace,
        trace_cores=trace_cores,
    )
    out = np.empty((B, S, HD), np.float32)
    for i in range(8):
        b, c = divmod(i, 4)
        o = res.results[i]["outp"]                           # [128, 8, 64]
        out[b, c * SS : (c + 1) * SS] = o.transpose(1, 0, 2).reshape(SS, HD)
    return out, res


def kernel(inputs, wq, wk, wv):
    out, _ = run_sharded(inputs, wq, wk, wv, trace=False)
    return out


# revision 9
# speedup vs baseline: 1.2457x; 1.1278x over previous
"""Sparse (sliding-window) attention head on 8 TRN2 NeuronCores.

Reference computation (B=2, S=4096, D=512, HD=64, SCALE=128):
    q = x @ wq ; k = x @ wk ; v = x @ wv          [B,S,64]
    scores[b,s,w] = q[b,s] . k[b,s-128+w] / 8     w in [0,256), zero-padded OOB
    out = softmax_w(scores) @ v_window            [B,S,64]

Sharding: 8 shards = (batch b, 1024-seq chunk c). Each shard gets a
zero-padded 128-halo of x on both sides, which reproduces the reference's
zero-padded (not masked) window semantics exactly. All compute is local.

v2 layout (per core):
    x arrives as 4 column-block tiles [128, 4dc, w] (bf16, host-transposed),
    DMA'd in consumption order on both HWDGE engines (Sync + Scalar).
    wq|wk packed into one [128,128] lhsT per d-chunk -> one PE pass emits
    qT (rows 0:64) and kT (rows 64:128) together into qkT [128, 1280].
    v stays natural [key,64] via per-chunk matmuls, packed pairwise in PSUM.
    Attention per 128-query block qb (starts as soon as its 384-col window
    of kT/qT/v is evacuated):
        scT[key,que] = kT_chunk.T @ qT_block      3 chunks of [128,128]
        ex = exp(scT/8) (ACT), em = ex*mask (DVE/Pool alternating)
        av4[:, qb%4, 0:65] += em_c.T @ vaug_c     ones col -> softmax denom
        norm: recip batched per 4 blocks, out_block = av*(1/denom)
    Output [128, 8, 64] partition-major, 2 DMAs; host unshuffles.
"""

import sys
import types

import numpy as np
import ml_dtypes

B, S, D = 2, 4096, 512
HD = 64
SCALE = 128
SS = S // 4          # 1024 positions per shard
HP = SCALE           # halo padding each side
NP = SS + 2 * HP     # 1280 padded positions
NKC = NP // 128      # 10 key chunks
NQB = SS // 128      # 8 query blocks
NDC = D // 128       # 4 d-chunks

# x column blocks (multiples of 128; qk proj segments == blocks)
BLOCKS = [(0, 384), (384, 384), (768, 256), (1024, 256)]

_CACHE = {}


def _ensure_hooks():
    """Register the axon NTFF profile hook; keep artifacts local."""
    if "antenv.axon_hooks" not in sys.modules:
        try:
            from trn_agent_boot.trn_boot import _ntff_profile_via_ctypes

            m = types.ModuleType("antenv.axon_hooks")
            m.get_axon_ntff_profile_hook = lambda: _ntff_profile_via_ctypes(
                "/opt/axon/libaxon_pjrt.so"
            )
            sys.modules["antenv.axon_hooks"] = m
        except Exception:
            pass
    import concourse.bass_utils as bass_utils

    bass_utils.upload_artifacts = lambda tmpdir: tmpdir


def _build_nc():
    import concourse.mybir as mybir
    import concourse.tile as tile
    from concourse import bacc

    bf = mybir.dt.bfloat16
    f32 = mybir.dt.float32
    AF = mybir.ActivationFunctionType

    nc = bacc.Bacc("TRN2", target_bir_lowering=False, debug=False, num_devices=8)

    x_d = nc.dram_tensor("x4", [128, NDC, NP], bf, kind="ExternalInput")
    w_d = nc.dram_tensor("wqkv", [128, NDC, 192], bf, kind="ExternalInput")
    mask_d = nc.dram_tensor("mask", [128, 384], bf, kind="ExternalInput")
    out_d = nc.dram_tensor("outp", [128, NQB, HD], f32, kind="ExternalOutput")

    # chunk -> block containing its 128 cols
    def blk_of(col):
        for bi, (s0, w) in enumerate(BLOCKS):
            if s0 <= col < s0 + w:
                return bi, col - s0
        raise AssertionError(col)

    with tile.TileContext(nc) as tc:
        with (
            tc.tile_pool(name="consts", bufs=1) as consts,
            tc.tile_pool(name="xtp", bufs=1) as xtp,
            tc.tile_pool(name="qkp", bufs=1) as qkp,
            tc.tile_pool(name="vgp", bufs=1) as vgp,
            tc.tile_pool(name="exp_p", bufs=2) as exp_p,
            tc.tile_pool(name="emp", bufs=2) as emp,
            tc.tile_pool(name="fin", bufs=2) as fin,
            tc.tile_pool(name="qkps", bufs=2, space="PSUM") as qkps,
            tc.tile_pool(name="vps", bufs=2, space="PSUM") as vps,
            tc.tile_pool(name="scps", bufs=2, space="PSUM") as scps,
            tc.tile_pool(name="avps", bufs=2, space="PSUM") as avps,
        ):
            # ---- DMAs first, split across both HWDGE engines ----
            xt = []
            for bi, (s0, w) in enumerate(BLOCKS):
                t = xtp.tile([128, NDC, w], bf, tag=f"xt{bi}")
                xt.append(t)
            w_s = consts.tile([128, NDC, 192], bf, tag="w")
            mask_s = consts.tile([128, 384], bf, tag="mask")

            nc.sync.dma_start(out=w_s, in_=w_d[:, :, :])
            nc.scalar.dma_start(out=xt[0], in_=x_d[:, :, 0:384])
            nc.sync.dma_start(out=xt[1], in_=x_d[:, :, 384:768])
            nc.scalar.dma_start(out=xt[2], in_=x_d[:, :, 768:1024])
            nc.sync.dma_start(out=xt[3], in_=x_d[:, :, 1024:1280])
            nc.scalar.dma_start(out=mask_s, in_=mask_d[:, :])

            # ---- memsets + ACT exp-table trigger ----
            zz = consts.tile([128, 1], f32, tag="zz")
            nc.gpsimd.memset(zz, 0.0)
            garb = consts.tile([128, 260], bf, tag="garb")
            nc.gpsimd.memset(garb, 0.5)
            vaug = vgp.tile([128, NKC, 66], bf, tag="vaug")
            nc.gpsimd.memset(vaug[:, :, 64:66], 1.0)
            ez = consts.tile([128, 1], f32, tag="ez")
            nc.scalar.activation(ez, zz, AF.Exp)

            qT_s = qkp.tile([64, SS], bf, tag="qT")
            kT_s = qkp.tile([64, NP], bf, tag="kT")
            ot = fin.tile([128, NQB, HD], f32, tag="ot")

            # ---- PE warmup: ramp to full clock while DMAs land ----
            for i in range(12):
                wp = avps.tile([128, 4, 65], f32, tag="av4")
                nc.tensor.matmul(
                    wp[:, :, :],
                    lhsT=garb[:, 0:128],
                    rhs=garb[:, :],
                    start=True,
                    stop=True,
                )

            # ---- helpers ----
            def qk_seg(bi):
                s0, w = BLOCKS[bi]
                ps = qkps.tile([128, 384], f32, tag="qkps")
                for dc in range(NDC):
                    nc.tensor.matmul(
                        ps[:, :w],
                        lhsT=w_s[:, dc, 0:128],
                        rhs=xt[bi][:, dc, :],
                        start=(dc == 0),
                        stop=(dc == NDC - 1),
                    )
                return ps, s0, w

            def v_chunk(kc, vp, j):
                bi, off = blk_of(kc * 128)
                for dc in range(NDC):
                    nc.tensor.matmul(
                        vp[:, j, :],
                        lhsT=xt[bi][:, dc, off : off + 128],
                        rhs=w_s[:, dc, 128:192],
                        start=(dc == 0),
                        stop=(dc == NDC - 1),
                    )

            def evac_seg(ps, s0, w, eng):
                # k rows (shifted down 64 partitions) + valid q cols
                cp = eng.copy if eng is nc.scalar else eng.tensor_copy
                cp(kT_s[:, s0 : s0 + w], ps[64:128, :w])
                qa, qb_ = max(s0, HP), min(s0 + w, HP + SS)
                if qa < qb_:
                    cp(qT_s[:, qa - HP : qb_ - HP], ps[0:64, qa - s0 : qb_ - s0])

            def sc_block(qb):
                sc = scps.tile([128, 384], f32, tag="sc")
                for c in range(3):
                    nc.tensor.matmul(
                        sc[:, c * 128 : (c + 1) * 128],
                        lhsT=kT_s[:, (qb + c) * 128 : (qb + c + 1) * 128],
                        rhs=qT_s[:, qb * 128 : (qb + 1) * 128],
                        start=True,
                        stop=True,
                    )
                return sc

            def exp_mask(sc, eng):
                ex = exp_p.tile([128, 384], bf, tag="ex")
                nc.scalar.activation(ex, sc, AF.Exp, scale=0.125)
                em = emp.tile([128, 384], bf, tag="em")
                eng.tensor_mul(em, ex, mask_s)
                return em

            def av_block(qb, em, av4, j):
                for c in range(3):
                    nc.tensor.matmul(
                        av4[:, j, :],
                        lhsT=em[:, c * 128 : (c + 1) * 128],
                        rhs=vaug[:, qb + c, 0:65],
                        start=(c == 0),
                        stop=(c == 2),
                    )

            # ---- pipeline (emission order == engine priority order) ----
            av4a = avps.tile([128, 4, 65], f32, tag="av4")

            ps0, s0, w0 = qk_seg(0)
            evac_seg(ps0, s0, w0, nc.scalar)          # ACT, before exp chain

            vp01 = vps.tile([128, 2, HD], f32, tag="vp")
            v_chunk(0, vp01, 0)
            v_chunk(1, vp01, 1)
            nc.vector.tensor_copy(vaug[:, 0:2, 0:64], vp01)
            vp23 = vps.tile([128, 2, HD], f32, tag="vp")
            v_chunk(2, vp23, 0)

            sc0 = sc_block(0)

            ps1, s1, w1 = qk_seg(1)
            evac_seg(ps1, s1, w1, nc.vector)

            v_chunk(3, vp23, 1)
            nc.vector.tensor_copy(vaug[:, 2:4, 0:64], vp23)

            em0 = exp_mask(sc0, nc.vector)
            sc1 = sc_block(1)
            av_block(0, em0, av4a, 0)
            em1 = exp_mask(sc1, nc.gpsimd)
            sc2 = sc_block(2)
            av_block(1, em1, av4a, 1)
            em2 = exp_mask(sc2, nc.gpsimd)

            vp45 = vps.tile([128, 2, HD], f32, tag="vp")
            v_chunk(4, vp45, 0)
            v_chunk(5, vp45, 1)
            nc.vector.tensor_copy(vaug[:, 4:6, 0:64], vp45)

            ps2, s2, w2 = qk_seg(2)
            evac_seg(ps2, s2, w2, nc.vector)

            av_block(2, em2, av4a, 2)
            sc3 = sc_block(3)
            em3 = exp_mask(sc3, nc.gpsimd)
            av_block(3, em3, av4a, 3)

            vp67 = vps.tile([128, 2, HD], f32, tag="vp")
            v_chunk(6, vp67, 0)
            v_chunk(7, vp67, 1)
            nc.vector.tensor_copy(vaug[:, 6:8, 0:64], vp67)

            ps3, s3, w3 = qk_seg(3)
            evac_seg(ps3, s3, w3, nc.vector)

            # group0 normalize + first output DMA
            rc0 = fin.tile([128, 4], f32, tag="rc")
            nc.vector.reciprocal(rc0, av4a[:, :, 64])
            for j in range(4):
                nc.vector.tensor_scalar_mul(
                    ot[:, j, :], av4a[:, j, 0:64], rc0[:, j : j + 1]
                )
            nc.sync.dma_start(out=out_d[:, 0:4, :], in_=ot[:, 0:4, :])

            av4b = avps.tile([128, 4, 65], f32, tag="av4")
            sc4 = sc_block(4)
            em4 = exp_mask(sc4, nc.gpsimd)
            av_block(4, em4, av4b, 0)

            vp89 = vps.tile([128, 2, HD], f32, tag="vp")
            v_chunk(8, vp89, 0)
            v_chunk(9, vp89, 1)
            nc.vector.tensor_copy(vaug[:, 8:10, 0:64], vp89)

            sc5 = sc_block(5)
            em5 = exp_mask(sc5, nc.gpsimd)
            av_block(5, em5, av4b, 1)
            rc45 = fin.tile([128, 2], f32, tag="rc2")
            nc.vector.reciprocal(rc45, av4b[:, 0:2, 64])
            nc.vector.tensor_scalar_mul(
                ot[:, 4, :], av4b[:, 0, 0:64], rc45[:, 0:1]
            )
            nc.vector.tensor_scalar_mul(
                ot[:, 5, :], av4b[:, 1, 0:64], rc45[:, 1:2]
            )

            sc6 = sc_block(6)
            em6 = exp_mask(sc6, nc.vector)
            av_block(6, em6, av4b, 2)
            rc6 = fin.tile([128, 1], f32, tag="rc1")
            nc.vector.reciprocal(rc6, av4b[:, 2, 64:65])
            nc.vector.tensor_scalar_mul(ot[:, 6, :], av4b[:, 2, 0:64], rc6)

            sc7 = sc_block(7)
            em7 = exp_mask(sc7, nc.vector)
            av_block(7, em7, av4b, 3)
            rc7 = fin.tile([128, 1], f32, tag="rc1")
            nc.vector.reciprocal(rc7, av4b[:, 3, 64:65])
            nc.vector.tensor_scalar_mul(ot[:, 7, :], av4b[:, 3, 0:64], rc7)

            nc.sync.dma_start(out=out_d[:, 4:8, :], in_=ot[:, 4:8, :])

    nc.compile()
    return nc


def _get_nc():
    if "nc" not in _CACHE:
        _ensure_hooks()
        _CACHE["nc"] = _build_nc()
    return _CACHE["nc"]


def _host_inputs(inputs, wq, wk, wv):
    bf16 = ml_dtypes.bfloat16
    x = np.asarray(inputs, dtype=np.float32)

    # wqkv[p, dc, 0:64]=wq, [64:128]=wk, [128:192]=wv  (rows dc*128+p)
    wcat = np.concatenate(
        [np.asarray(wq), np.asarray(wk), np.asarray(wv)], axis=1
    ).astype(np.float32)                                     # [512, 192]
    wqkv = np.ascontiguousarray(
        wcat.reshape(NDC, 128, 192).transpose(1, 0, 2)
    ).astype(bf16)                                           # [128, 4, 192]

    p = np.arange(128)[:, None]
    q = np.arange(128)[None, :]
    mask = np.concatenate(
        [(p >= q), np.ones((128, 128), bool), (p < q)], axis=1
    ).astype(bf16)                                           # [128, 384]

    in_maps = []
    for i in range(8):
        b, c = divmod(i, 4)
        s0 = c * SS
        xp = np.zeros((NP, D), np.float32)
        lo = max(0, s0 - HP)
        hi = min(S, s0 + SS + HP)
        xp[lo - (s0 - HP) : hi - (s0 - HP)] = x[b, lo:hi]
        x4 = np.ascontiguousarray(
            xp.T.reshape(NDC, 128, NP).transpose(1, 0, 2)
        ).astype(bf16)                                       # [128, 4, 1280]
        in_maps.append({"x4": x4, "wqkv": wqkv, "mask": mask})
    return in_maps


def run_sharded(inputs, wq, wk, wv, trace=False, trace_cores=None):
    """Run the SPMD kernel; returns (out [B,S,HD] f32, BassKernelResults)."""
    _ensure_hooks()
    import concourse.bass_utils as bass_utils

    nc = _get_nc()
    in_maps = _host_inputs(inputs, wq, wk, wv)
    res = bass_utils.run_bass_kernel_spmd(
        nc,
        in_maps,
        core_ids=list(range(8)),
        trace=trace,
        trace_cores=trace_cores,
    )
    out = np.empty((B, S, HD), np.float32)
    for i in range(8):
        b, c = divmod(i, 4)
        o = res.results[i]["outp"]                           # [128, 8, 64]
        out[b, c * SS : (c + 1) * SS] = o.transpose(1, 0, 2).reshape(SS, HD)
    return out, res


def kernel(inputs, wq, wk, wv):
    out, _ = run_sharded(inputs, wq, wk, wv, trace=False)
    return out
